# revision 1
# baseline (speedup 1.0000x reference)
"""Multi-head attention (B=4, L=2048, D=1024, H=16) on 8 TRN2 NeuronCores.

Sharding: core c handles batch b=c//2, query half qh=c%2 (1024 query tokens,
all heads, full 2048-key context). K/V projections are duplicated across the
2 cores sharing a batch; no cross-core communication needed.

Per-core dataflow (all matmul operands bf16, PSUM fp32):
  - Q.T = (Wq.T).T-chunks @ x_q.T       -> [1024 dk, 1024 q]   (transposed)
  - K.T likewise                        -> [1024 dk, 2048 k]
  - V   = (x_k.T).T-chunks @ Wv.T       -> [2048 k, 1024 dk]   (+ ones col/head)
  - per head: S.T[k,q] = K_h.T.T @ Q_h.T ; E = exp(S.T/8) (ScalarE)
              E *= mask.T (VectorE) ; AV.T[65,q] += V_h'.T.T @ E  (PE)
              row 64 of AV.T = softmax denominators (ones column trick)
              C.T[dk,q] = AV.T[0:64] * bcast(1/denom)
  - out[q,1024] = C.T.T-chunks @ Wo.T + bo
"""

import sys
import functools

sys.path.insert(0, "/opt/trn_rl_repo")

import numpy as np
import ml_dtypes

BF16NP = ml_dtypes.bfloat16

B, L, D, H, DK = 4, 2048, 1024, 16, 64
NCORES = 8
LQ = L // 2          # query tokens per core
NI = D // 128        # input-dim chunks
NM = D // 128        # dk-dim tiles
NJ = L // 128        # key tiles
VW = H * (DK + 1)    # V tile width incl. per-head ones column (1040)


def _build():
    import concourse.mybir as mybir
    import concourse.tile as tile
    from concourse import bacc

    dt = mybir.dt
    F32, BF = dt.float32, dt.bfloat16
    AF = mybir.ActivationFunctionType

    nc = bacc.Bacc("TRN2", target_bir_lowering=False, debug=False,
                   num_devices=NCORES)

    xq_d = nc.dram_tensor("xq", [NI, 128, LQ], BF, kind="ExternalInput")
    xk_d = nc.dram_tensor("xk", [NI, 128, L], BF, kind="ExternalInput")
    xv_d = nc.dram_tensor("xv", [NI, 128, L], BF, kind="ExternalInput")
    wq_d = nc.dram_tensor("wq", [NI, 128, D], BF, kind="ExternalInput")
    wk_d = nc.dram_tensor("wk", [NI, 128, D], BF, kind="ExternalInput")
    wv_d = nc.dram_tensor("wv", [NI, 128, D], BF, kind="ExternalInput")
    wo_d = nc.dram_tensor("wo", [NI, 128, D], BF, kind="ExternalInput")
    mt_d = nc.dram_tensor("maskt", [NJ, 128, LQ], BF, kind="ExternalInput")
    bq_d = nc.dram_tensor("bqt", [128, NM], F32, kind="ExternalInput")
    bk_d = nc.dram_tensor("bkt", [128, NM], F32, kind="ExternalInput")
    bo_d = nc.dram_tensor("bor", [1, D], BF, kind="ExternalInput")
    out_d = nc.dram_tensor("out", [NM, 128, D], F32, kind="ExternalOutput")

    keep = []  # keep single-tile pools' free-closures alive (GC releases pools)

    def single(shape, dtyp, name):
        t, free = tc.tile(shape, dtyp, name=name)
        keep.append(free)
        return t

    with tile.TileContext(nc) as tc:
        # ---- persistent tiles ----
        QT = [single([128, LQ], BF, f"qt{m}") for m in range(NM)]
        KT = [single([128, L], BF, f"kt{m}") for m in range(NM)]
        VP = [single([128, VW], BF, f"vp{j}") for j in range(NJ)]
        CT = [single([128, LQ], BF, f"ct{m}") for m in range(NM)]
        MT = [single([128, LQ], BF, f"mt{j}") for j in range(NJ)]
        bq_sb = single([128, NM], F32, "bq_sb")
        bk_sb = single([128, NM], F32, "bk_sb")
        bo_sb = single([1, D], BF, "bo_sb")
        ones_sb = single([1, 128], BF, "ones_sb")

        nc.vector.memset(ones_sb[:], 1.0)

        with (
            tc.tile_pool(name="wp", bufs=10) as wp,
            tc.tile_pool(name="xp", bufs=16) as xp,
            tc.tile_pool(name="ep", bufs=6) as ep,
            tc.tile_pool(name="bp", bufs=2) as bp,
            tc.tile_pool(name="fp", bufs=2) as fp,
            tc.tile_pool(name="sp", bufs=2, space="PSUM") as sp,
            tc.tile_pool(name="avp", bufs=3, space="PSUM") as avp,
            tc.tile_pool(name="kp", bufs=1, space="PSUM") as kp,
        ):
            # ---------- Q.T / K.T projections ----------
            def qk_proj(name, w_dram, x_dram, ltok, dst, bias, ms=range(NM),
                        ws=None):
                if ws is None:
                    ws = []
                    xs0 = []
                    for i in range(NI):
                        wt = wp.tile([128, D], BF, tag="w", name=f"w{name}{i}")
                        nc.sync.dma_start(wt[:], w_dram.ap()[i])
                        ws.append(wt)
                        xt = xp.tile([128, 512], BF, tag="x", name=f"x{name}0_{i}")
                        nc.sync.dma_start(xt[:], x_dram.ap()[i, :, 0:512])
                        xs0.append(xt)
                else:
                    xs0 = None
                if name == "q":
                    nc.sync.dma_start(bq_sb[:], bq_d.ap())
                    nc.sync.dma_start(bk_sb[:], bk_d.ap())
                    nc.sync.dma_start(bo_sb[:], bo_d.ap())
                for c in range(ltok // 512):
                    if c == 0 and xs0 is not None:
                        xs = xs0
                    else:
                        xs = []
                        for i in range(NI):
                            xt = xp.tile([128, 512], BF, tag="x", name=f"x{name}{c}_{i}")
                            nc.sync.dma_start(xt[:], x_dram.ap()[i, :, c * 512:(c + 1) * 512])
                            xs.append(xt)
                    for m in ms:
                        ps = sp.tile([128, 512], F32, tag="s", name=f"ps{name}{c}{m}")
                        for i in range(NI):
                            nc.tensor.matmul(
                                ps[:], ws[i][:, m * 128:(m + 1) * 128], xs[i][:],
                                start=(i == 0), stop=(i == NI - 1))
                        nc.scalar.activation(
                            dst[m][:, c * 512:(c + 1) * 512], ps[:], AF.Identity,
                            bias=bias[:, m:m + 1], scale=1.0)
                return ws

            def k_late_units(ws):
                # m 4..7 of K-proj as (c, m) units: x chunk loaded per c,
                # PSUM drained by VectorE (ScalarE is the attention bottleneck)
                for c in range(L // 512):
                    xs = []
                    for i in range(NI):
                        xt = xp.tile([128, 512], BF, tag="x", name=f"xk2{c}_{i}")
                        nc.sync.dma_start(
                            xt[:], xk_d.ap()[i, :, c * 512:(c + 1) * 512])
                        xs.append(xt)
                    for m in range(NM // 2, NM):
                        def unit(c=c, m=m, xs=xs):
                            ps = kp.tile([128, 512], F32, tag="k",
                                         name=f"psk2{c}{m}")
                            for i in range(NI):
                                nc.tensor.matmul(
                                    ps[:], ws[i][:, m * 128:(m + 1) * 128],
                                    xs[i][:], start=(i == 0), stop=(i == NI - 1))
                            nc.vector.tensor_scalar_add(
                                KT[m][:, c * 512:(c + 1) * 512], ps[:],
                                bk_sb[:, m:m + 1])
                        yield unit

            qk_proj("q", wq_d, xq_d, LQ, QT, bq_sb)

            # ---------- V projection (untransposed, + ones columns) ----------
            wvs = []
            for i in range(NI):
                wt = wp.tile([128, D], BF, tag="w", name=f"wv{i}")
                nc.sync.dma_start(wt[:], wv_d.ap()[i])
                wvs.append(wt)
            for c in range(L // 512):
                xs = []
                for i in range(NI):
                    xt = xp.tile([128, 512], BF, tag="x", name=f"xv{c}_{i}")
                    nc.sync.dma_start(xt[:], xv_d.ap()[i, :, c * 512:(c + 1) * 512])
                    xs.append(xt)
                for jj in range(4):
                    j = c * 4 + jj
                    ps = sp.tile([128, D], F32, tag="s", name=f"psv{j}")
                    for half in range(2):
                        hs = slice(half * 512, half * 512 + 512)
                        for i in range(NI):
                            nc.tensor.matmul(
                                ps[:, hs],
                                xs[i][:, jj * 128:(jj + 1) * 128],
                                wvs[i][:, hs],
                                start=(i == 0), stop=(i == NI - 1))
                    # strided copy into per-head 65-wide slots
                    dst = VP[j][:].rearrange("p (h w) -> p h w", w=DK + 1)[:, :, 0:DK]
                    src = ps[:].rearrange("p (h w) -> p h w", w=DK)
                    nc.vector.tensor_copy(dst, src)
                    nc.vector.memset(VP[j][:, DK::DK + 1], 1.0)

            for j in range(NJ):
                nc.sync.dma_start(MT[j][:], mt_d.ap()[j])
            wks = qk_proj("k", wk_d, xk_d, L, KT, bk_sb, ms=range(NM // 2))
            k_units = list(k_late_units(wks))

            # ---------- load Wo while attention runs ----------
            wos = []
            for i in range(NI):
                wt = wp.tile([128, D], BF, tag="w", name=f"wo{i}")
                nc.sync.dma_start(wt[:], wo_d.ap()[i])
                wos.append(wt)

            # ---------- attention ----------
            for h in range(H):
                if 0 < h <= 8:
                    for u in k_units[(h - 1) * 2:h * 2]:
                        u()
                m, off = h // 2, (h % 2) * DK
                avh = [avp.tile([DK + 1, 512], F32, tag="av",
                                name=f"av{h}_{x}") for x in range(2)]
                for j in range(NJ):
                    s = sp.tile([128, LQ], F32, tag="s", name=f"s{h}_{j}")
                    for half in range(2):
                        hs = slice(half * 512, half * 512 + 512)
                        nc.tensor.matmul(
                            s[:, hs],
                            KT[m][off:off + DK, j * 128:(j + 1) * 128],
                            QT[m][off:off + DK, hs],
                            start=True, stop=True)
                    e = ep.tile([128, LQ], BF, tag="e", name=f"e{h}_{j}")
                    nc.scalar.activation(e[:], s[:], AF.Exp, scale=0.125)
                    nc.vector.tensor_mul(e[:], e[:], MT[j][:])
                    for half in range(2):
                        hs = slice(half * 512, half * 512 + 512)
                        nc.tensor.matmul(
                            avh[half][:],
                            VP[j][:, h * (DK + 1):(h + 1) * (DK + 1)],
                            e[:, hs],
                            start=(j == 0), stop=(j == NJ - 1))
                bc = bp.tile([DK, LQ], F32, tag="b", name=f"bc{h}")
                for half in range(2):
                    hs = slice(half * 512, half * 512 + 512)
                    nc.vector.reciprocal(bc[0:1, hs], avh[half][DK:DK + 1, :])
                nc.gpsimd.partition_broadcast(bc[:], bc[0:1, :], channels=DK)
                for half in range(2):
                    hs = slice(half * 512, half * 512 + 512)
                    nc.vector.tensor_mul(
                        CT[m][off:off + DK, hs], avh[half][0:DK, :], bc[:, hs])

            # ---------- output projection ----------
            for t in range(NM):
                po = sp.tile([128, D], F32, tag="s", name=f"po{t}")
                for half in range(2):
                    hs = slice(half * 512, half * 512 + 512)
                    for c in range(NI):
                        nc.tensor.matmul(
                            po[:, hs],
                            CT[c][:, t * 128:(t + 1) * 128],
                            wos[c][:, hs],
                            start=(c == 0), stop=False)
                for half in range(2):
                    hs = slice(half * 512, half * 512 + 512)
                    nc.tensor.matmul(po[:, hs], ones_sb[:], bo_sb[:, hs],
                                     start=False, stop=True)
                f = fp.tile([128, D], F32, tag="f", name=f"f{t}")
                nc.scalar.activation(f[:], po[:], AF.Copy)
                nc.sync.dma_start(out_d.ap()[t], f[:])

    nc.compile()
    nc._keep_tile_frees = keep
    return nc


@functools.lru_cache(maxsize=1)
def _built():
    return _build()


def _prep_core(c, q, k, v, mask01T, wqt, wkt, wvt, wot, bqt, bkt, bor):
    b, qh = c // 2, c % 2
    qs = slice(qh * LQ, (qh + 1) * LQ)
    xq = np.ascontiguousarray(q[b, qs, :].T).astype(BF16NP).reshape(NI, 128, LQ)
    xk = np.ascontiguousarray(k[b].T).astype(BF16NP).reshape(NI, 128, L)
    xv = np.ascontiguousarray(v[b].T).astype(BF16NP).reshape(NI, 128, L)
    maskt = np.ascontiguousarray(mask01T[:, qs]).reshape(NJ, 128, LQ)
    return {
        "xq": xq, "xk": xk, "xv": xv,
        "wq": wqt, "wk": wkt, "wv": wvt, "wo": wot,
        "maskt": maskt, "bqt": bqt, "bkt": bkt, "bor": bor,
    }


def kernel(q, k, v, attn_mask, Wq, bq, Wk, bk, Wv, bv, Wo, bo):
    from concourse import bass_utils

    nc = _built()

    q = np.asarray(q, np.float32)
    k = np.asarray(k, np.float32)
    v = np.asarray(v, np.float32)
    wqt = np.ascontiguousarray(np.asarray(Wq, np.float32).T).astype(BF16NP).reshape(NI, 128, D)
    wkt = np.ascontiguousarray(np.asarray(Wk, np.float32).T).astype(BF16NP).reshape(NI, 128, D)
    wvt = np.ascontiguousarray(np.asarray(Wv, np.float32).T).astype(BF16NP).reshape(NI, 128, D)
    wot = np.ascontiguousarray(np.asarray(Wo, np.float32).T).astype(BF16NP).reshape(NI, 128, D)
    mask01T = np.ascontiguousarray((np.asarray(attn_mask)[0, 0] != 0).T.astype(BF16NP))
    bqt = np.ascontiguousarray(np.asarray(bq, np.float32).reshape(NM, 128).T)
    bkt = np.ascontiguousarray(np.asarray(bk, np.float32).reshape(NM, 128).T)
    bo_eff = np.asarray(bo, np.float32) + np.asarray(Wo, np.float32) @ np.asarray(bv, np.float32)
    bor = bo_eff.astype(BF16NP).reshape(1, D)

    in_maps = [
        _prep_core(c, q, k, v, mask01T, wqt, wkt, wvt, wot, bqt, bkt, bor)
        for c in range(NCORES)
    ]
    res = bass_utils.run_bass_kernel_spmd(nc, in_maps, core_ids=list(range(NCORES)))

    out = np.empty((B, L, D), np.float32)
    for c in range(NCORES):
        b, qh = c // 2, c % 2
        out[b, qh * LQ:(qh + 1) * LQ, :] = res.results[c]["out"].reshape(LQ, D)
    return out

